# revision 1
# baseline (speedup 1.0000x reference)
"""LlamaAttention (B=2, S=2048, H=4096, NH=32) on 8 Trainium2 NeuronCores.

Sharding: tensor-parallel over heads (4 heads / core). Column-parallel
Wq/Wk/Wv, row-parallel Wo; the Wo partial sums are reduced on the host
(the all-reduce of the TP recipe, done during unshard).

Per-core dataflow (all matmuls fp32r = full-rate reduced-precision fp32):
  per batch b:
    phase 1: Q^T,K^T = RoPE(W^T-chunk @ X^T-chunk) -> DRAM  [d, t] layout
             V       = X^T-chunk^T @ WvT           -> DRAM  [t, d] layout
    phase 2: per head: S^T[k,q] = K^T-tile^T @ Q^T  (contraction d)
             exp on ACT; denominators via ones-matmul (partition-broadcast
             column sums); ctx^T[d,q] = V-tile^T @ expS^T over k tiles.
             Only non-fully-masked 128x512 score blocks are computed.
  phase 3: O^T partial = WoT-tile^T @ ctx^T -> DRAM [o, t] layout

Host side: pre-transposes X and the weights (layout marshaling), builds
the block structure from the attention mask, sums the 8 partial O^T
outputs and transposes back.
"""
import sys

sys.path.insert(0, "/opt/trn_rl_repo")

import numpy as np

import concourse.bass as bass
import concourse.bacc as bacc
import concourse.tile as tile
import concourse.mybir as mybir

B, S, H, NH = 2, 2048, 4096, 32
HD = H // NH          # 128
NC = 8                # cores
DL = H // NC          # 512 local dims (4 heads / core)
NHL = NH // NC        # 4 local heads
BT = B * S            # 4096 tokens
P = 128
SLICE = 1024          # phase-1 token slice (W chunks reused across it)
CH = 512              # phase-1 X^T chunk (matmul moving dim)
QT = 512              # phase-2 query tile (free dim)
KT = 128              # phase-2 key tile (partition dim)
NKO = H // P          # 32 contraction subtiles

DT = mybir.dt.float32
DTR = mybir.dt.float32r
F32 = mybir.dt.float32
AF = mybir.ActivationFunctionType


def _phase1_batch(nc, tc, b, pools, aps, scratch):
    """QKV projections + RoPE for batch b."""
    p1, p1t, p1w, p1s, p1r, psA, psV = pools
    xt3, wq3, wk3, wv3, cosq, sinq, cosk, sink = aps
    qt_d, kt_d, v_d = scratch          # per-batch tiles [DL, S], [DL, S], [S, DL]

    for sl in range(S // SLICE):                       # 2 slices per batch
        t0 = b * S + sl * SLICE                        # global token offset
        xch = []
        for c in range(SLICE // CH):                   # 2 chunks
            xc = p1.tile([P, NKO, CH], DTR, tag="xt", name=f"xt{c}")
            nc.sync.dma_start(xc[:], xt3[:, :, bass.ds(t0 + c * CH, CH)])
            xch.append(xc)
        tabs = {}
        for nm, t_ap in (("cq", cosq), ("sq", sinq), ("ck", cosk), ("sk", sink)):
            tt = p1t.tile([P, SLICE], DT, tag="tab_" + nm)
            nc.sync.dma_start(tt[:], t_ap[:, bass.ds(sl * SLICE, SLICE)])
            tabs[nm] = tt
        # --- Q^T and K^T with RoPE ---
        for (w3, cnm, snm, outd) in ((wq3, "cq", "sq", qt_d),
                                     (wk3, "ck", "sk", kt_d)):
            cosT, sinT = tabs[cnm], tabs[snm]
            for dsub in range(DL // P):
                w_sb = p1w.tile([P, NKO, P], DTR, tag="wqk")
                nc.sync.dma_start(w_sb[:], w3[:, :, bass.ts(dsub, P)])
                for c in range(SLICE // CH):
                    psum = psA.tile([P, CH], F32, tag="qk")
                    for hs in range(NKO):
                        nc.tensor.matmul(
                            psum[:], w_sb[:, hs, :], xch[c][:, hs, :],
                            start=(hs == 0), stop=(hs == NKO - 1))
                    csl = bass.ds(sl * SLICE + c * CH, CH)
                    tsl = bass.ds(c * CH, CH)
                    rc = p1r.tile([P, CH], DTR, tag="rc")
                    rs = p1r.tile([P, CH], F32, tag="rs")
                    nc.vector.tensor_mul(rc[:], psum[:], cosT[:, tsl])
                    nc.vector.tensor_mul(
                        rs[0:64, :], psum[64:128, :], sinT[0:64, tsl])
                    nc.vector.tensor_mul(
                        rs[64:128, :], psum[0:64, :], sinT[64:128, tsl])
                    nc.vector.tensor_tensor(
                        rc[0:64, :], rc[0:64, :], rs[0:64, :],
                        mybir.AluOpType.subtract)
                    nc.vector.tensor_tensor(
                        rc[64:128, :], rc[64:128, :], rs[64:128, :],
                        mybir.AluOpType.add)
                    nc.sync.dma_start(outd[bass.ts(dsub, P), csl], rc[:])
        # --- V in [t, d] layout; waves in reverse chunk order so the
        # first chunk's slot frees early for the next slice's prefetch ---
        for c in reversed(range(SLICE // CH)):
            psums = [psV.tile([P, DL], F32, tag="v", name=f"vps{j}")
                     for j in range(CH // P)]
            for hs in range(NKO):
                wv_sb = p1s.tile([P, DL], DTR, tag="wv")
                nc.sync.dma_start(wv_sb[:], wv3[:, hs, :])
                for j in range(CH // P):
                    nc.tensor.matmul(
                        psums[j][:], xch[c][:, hs, bass.ts(j, P)], wv_sb[:],
                        start=(hs == 0), stop=(hs == NKO - 1))
            for j in range(CH // P):
                vo = p1s.tile([P, DL], DTR, tag="vo")
                nc.vector.tensor_copy(vo[:], psums[j][:])
                nc.sync.dma_start(
                    v_d[bass.ds(sl * SLICE + c * CH + j * P, P), :], vo[:])


def _phase2_batch(nc, tc, b, spec, pools, maskt, mb, ones_r, scratch, ctxT):
    """Attention for batch b -> ctxT [P, NHL, S]."""
    p2, p2e, p2m, psS, psSum, psC = pools
    qt_d, kt_d, v_d = scratch

    for h in range(NHL):
        k_sb = p2.tile([P, S], DTR, tag="k_sb")
        nc.sync.dma_start(k_sb[:], kt_d[bass.ts(h, P), :])
        q_sb = p2.tile([P, S], DTR, tag="q_sb")
        nc.sync.dma_start(q_sb[:], qt_d[bass.ts(h, P), :])
        v_sb = p2.tile([P, S // P, P], DTR, tag="v_sb")
        nc.sync.dma_start(
            v_sb[:], v_d[:, bass.ts(h, P)].rearrange("(kt p) d -> p kt d", p=P))
        for qt in range(S // QT):
            blocks = spec[qt]
            nb = len(blocks)
            psum_sum = psSum.tile([P, QT], F32, tag="sum")
            psum_ctx = psC.tile([P, QT], F32, tag="ctx")
            for bi, (kt, masked) in enumerate(blocks):
                psum_s = psS.tile([P, QT], F32, tag="s")
                nc.tensor.matmul(
                    psum_s[:], k_sb[:, bass.ts(kt, KT)],
                    q_sb[:, bass.ts(qt, QT)], start=True, stop=True)
                if masked:
                    mk = p2m.tile([P, QT], DT, tag="mk")
                    nc.sync.dma_start(
                        mk[:], maskt[mb, bass.ts(kt, KT), bass.ts(qt, QT)])
                    nc.vector.tensor_tensor(
                        psum_s[:], psum_s[:], mk[:], mybir.AluOpType.add)
                e_sb = p2e.tile([P, QT], DTR, tag="e")
                nc.scalar.activation(e_sb[:], psum_s[:], AF.Exp)
                nc.tensor.matmul(psum_sum[:], ones_r[:], e_sb[:],
                                 start=(bi == 0), stop=(bi == nb - 1))
                nc.tensor.matmul(psum_ctx[:], v_sb[:, kt, :], e_sb[:],
                                 start=(bi == 0), stop=(bi == nb - 1))
            recip = p2e.tile([P, QT], F32, tag="recip")
            nc.vector.reciprocal(recip[:], psum_sum[:])
            nc.vector.tensor_mul(
                ctxT[:, h, bass.ts(qt, QT)], psum_ctx[:], recip[:])


def _phase3(nc, tc, pools, wo3, ctx_tiles, ot):
    p3w, p3o, psO = pools
    for b in range(B):
        ctxT = ctx_tiles[b]
        for oi in range(H // P):
            wo_sb = p3w.tile([P, NHL, P], DTR, tag="wo")
            nc.sync.dma_start(wo_sb[:], wo3[:, :, bass.ts(oi, P)])
            for qt in range(S // QT):
                psum_o = psO.tile([P, QT], F32, tag="o")
                for hs in range(NHL):
                    nc.tensor.matmul(
                        psum_o[:], wo_sb[:, hs, :], ctxT[:, hs, bass.ts(qt, QT)],
                        start=(hs == 0), stop=(hs == NHL - 1))
                o_sb = p3o.tile([P, QT], DT, tag="o_sb")
                nc.vector.tensor_copy(o_sb[:], psum_o[:])
                nc.sync.dma_start(
                    ot[bass.ts(oi, P), bass.ds(b * S + qt * QT, QT)], o_sb[:])


def _build(specs, n_mb, reps=1, phases=(1, 2, 3)):
    nc = bacc.Bacc()

    xt = nc.declare_dram_parameter("xt", [H, BT], DTR, isOutput=False)
    wqt = nc.declare_dram_parameter("wqt", [H, DL], DTR, isOutput=False)
    wkt = nc.declare_dram_parameter("wkt", [H, DL], DTR, isOutput=False)
    wvt = nc.declare_dram_parameter("wvt", [H, DL], DTR, isOutput=False)
    wot = nc.declare_dram_parameter("wot", [DL, H], DTR, isOutput=False)
    maskt = nc.declare_dram_parameter("maskt", [n_mb, S, S], DT, isOutput=False)
    cosq = nc.declare_dram_parameter("cosq", [HD, S], DT, isOutput=False)
    sinq = nc.declare_dram_parameter("sinq", [HD, S], DT, isOutput=False)
    cosk = nc.declare_dram_parameter("cosk", [HD, S], DT, isOutput=False)
    sink = nc.declare_dram_parameter("sink", [HD, S], DT, isOutput=False)
    ot = nc.declare_dram_parameter("ot", [H, BT], DT, isOutput=True)

    xt3 = xt.rearrange("(ho p) t -> p ho t", p=P)
    wq3 = wqt.rearrange("(ho p) d -> p ho d", p=P)
    wk3 = wkt.rearrange("(ho p) d -> p ho d", p=P)
    wv3 = wvt.rearrange("(ho p) d -> p ho d", p=P)
    wo3 = wot.rearrange("(hs p) o -> p hs o", p=P)

    import contextlib

    with tile.TileContext(nc) as tc:
        with (
            tc.tile_pool(name="glob", bufs=1) as glob,
            tc.tile_pool(name="dram", bufs=1, space="DRAM") as dram,
        ):
            scratches = []
            for b in range(B):
                qd = dram.tile([DL, S], DTR, tag=f"qt_d{b}", name=f"qt_d{b}")
                kd = dram.tile([DL, S], DTR, tag=f"kt_d{b}", name=f"kt_d{b}")
                vd = dram.tile([S, DL], DTR, tag=f"v_d{b}", name=f"v_d{b}")
                scratches.append((qd, kd, vd))

            ones_f = glob.tile([P, P], F32, tag="ones_f")
            nc.any.memset(ones_f[:], 1.0)
            ones_r = glob.tile([P, P], DTR, tag="ones_r")
            nc.vector.tensor_copy(ones_r[:], ones_f[:])

            loop_cm = tc.For_i(0, reps, 1) if reps > 1 else contextlib.nullcontext()
            with loop_cm:
                aps = (xt3, wq3, wk3, wv3, cosq, sinq, cosk, sink)
                if 1 in phases:
                    with (
                        tc.tile_pool(name="p1", bufs=2) as p1,
                        tc.tile_pool(name="p1t", bufs=1) as p1t,
                        tc.tile_pool(name="p1w", bufs=2) as p1w,
                        tc.tile_pool(name="p1s", bufs=3) as p1s,
                        tc.tile_pool(name="p1r", bufs=2) as p1r,
                        tc.tile_pool(name="psA", bufs=2, space="PSUM") as psA,
                        tc.tile_pool(name="psV", bufs=4, space="PSUM") as psV,
                    ):
                        p1pools = (p1, p1t, p1w, p1s, p1r, psA, psV)
                        for b in range(B):
                            _phase1_batch(nc, tc, b, p1pools, aps, scratches[b])
                if 2 in phases:
                    with tc.tile_pool(name="ctxp", bufs=1) as ctxp:
                        ctx_tiles = []
                        with (
                            tc.tile_pool(name="p2", bufs=2) as p2,
                            tc.tile_pool(name="p2e", bufs=3) as p2e,
                            tc.tile_pool(name="p2m", bufs=2) as p2m,
                            tc.tile_pool(name="psS", bufs=3, space="PSUM") as psS,
                            tc.tile_pool(name="psSum", bufs=2, space="PSUM") as psSum,
                            tc.tile_pool(name="psC", bufs=2, space="PSUM") as psC,
                        ):
                            for b in range(B):
                                mb = b % n_mb
                                ctxT = ctxp.tile([P, NHL, S], DTR, tag=f"ctxT{b}",
                                                 name=f"ctxT{b}")
                                ctx_tiles.append(ctxT)
                                _phase2_batch(
                                    nc, tc, b, specs[mb],
                                    (p2, p2e, p2m, psS, psSum, psC),
                                    maskt, mb, ones_r, scratches[b], ctxT)
                        if 3 in phases:
                            with (
                                tc.tile_pool(name="p3w", bufs=3) as p3w,
                                tc.tile_pool(name="p3o", bufs=4) as p3o,
                                tc.tile_pool(name="psO", bufs=4, space="PSUM") as psO,
                            ):
                                _phase3(nc, tc, (p3w, p3o, psO), wo3, ctx_tiles, ot)
    nc.finalize()
    return nc


def _rope_tables():
    inv_freq = 1.0 / (10000.0 ** (np.arange(0, HD, 2, dtype=np.float32) / HD))
    t = np.arange(S, dtype=np.float32)
    freqs = np.einsum("i,j->ij", t, inv_freq)
    emb = np.concatenate([freqs, freqs], axis=-1)        # [S, HD]
    return np.cos(emb).astype(np.float32), np.sin(emb).astype(np.float32)


def _block_spec(mask):
    """mask: [S, S] additive mask (q, k). Returns per-qt list of (kt, masked)."""
    spec = []
    for qt in range(S // QT):
        row = []
        sub_q = mask[qt * QT:(qt + 1) * QT]
        for kt in range(S // KT):
            blk = sub_q[:, kt * KT:(kt + 1) * KT]
            if np.all(blk <= -1e8):
                continue                        # fully masked -> skip
            masked = bool(np.any(blk != 0.0))
            row.append((kt, masked))
        assert row, "a query tile with all keys masked is not supported"
        spec.append(row)
    return spec


_CACHE = {}


def kernel(hidden_states, attention_mask, Wq, Wk, Wv, Wo):
    from concourse.bass_utils import run_bass_kernel_spmd

    hidden_states = np.asarray(hidden_states, dtype=np.float32)
    attention_mask = np.asarray(attention_mask, dtype=np.float32)
    Wq = np.asarray(Wq, dtype=np.float32)
    Wk = np.asarray(Wk, dtype=np.float32)
    Wv = np.asarray(Wv, dtype=np.float32)
    Wo = np.asarray(Wo, dtype=np.float32)

    xt = np.ascontiguousarray(hidden_states.reshape(BT, H).T)   # [H, BT]
    wqT = np.ascontiguousarray(Wq.T)                            # [H, H] (in, out)
    wkT = np.ascontiguousarray(Wk.T)
    wvT = np.ascontiguousarray(Wv.T)
    woT = np.ascontiguousarray(Wo.T)                            # [H(in'), H(out)]

    masks = attention_mask[:, 0]                                # [B, S, S]
    same = bool(np.array_equal(masks[0], masks[1])) if B == 2 else True
    n_mb = 1 if same else B
    specs = [_block_spec(masks[i]) for i in range(n_mb)]
    maskt = np.ascontiguousarray(
        np.stack([masks[i].T for i in range(n_mb)]))            # [n_mb, S(k), S(q)]

    cos, sin = _rope_tables()
    scale = 1.0 / np.sqrt(np.float32(HD))
    cosq = np.ascontiguousarray((cos * scale).T)                # [HD, S]
    sinq = np.ascontiguousarray((sin * scale).T)
    cosk = np.ascontiguousarray(cos.T)
    sink = np.ascontiguousarray(sin.T)

    key = (n_mb, tuple(tuple(map(tuple, s)) for s in specs))
    if key not in _CACHE:
        _CACHE[key] = _build(specs, n_mb)
    nc = _CACHE[key]

    in_maps = []
    for g in range(NC):
        dsl = slice(g * DL, (g + 1) * DL)
        in_maps.append({
            "xt": xt,
            "wqt": np.ascontiguousarray(wqT[:, dsl]),
            "wkt": np.ascontiguousarray(wkT[:, dsl]),
            "wvt": np.ascontiguousarray(wvT[:, dsl]),
            "wot": np.ascontiguousarray(woT[dsl, :]),
            "maskt": maskt,
            "cosq": cosq, "sinq": sinq, "cosk": cosk, "sink": sink,
        })

    try:
        res = run_bass_kernel_spmd(nc, in_maps, list(range(NC)), trace=False)
    except Exception:
        # one retry: a wedged NeuronCore usually recovers on re-dispatch
        import time as _time
        _time.sleep(5)
        res = run_bass_kernel_spmd(nc, in_maps, list(range(NC)), trace=False)
    acc = np.zeros((H, BT), dtype=np.float32)
    for g in range(NC):
        acc += res.results[g]["ot"]
    return np.ascontiguousarray(acc.T).reshape(B, S, H)



# revision 7
# speedup vs baseline: 56.1925x; 56.1925x over previous
"""LlamaAttention (B=2, S=2048, H=4096, NH=32) on 8 Trainium2 NeuronCores.

Sharding: tensor-parallel over heads (4 heads / core). Column-parallel
Wq/Wk/Wv, row-parallel Wo; the Wo partial sums are reduced on the host
(the all-reduce of the TP recipe, done during unshard).

v2 dataflow (PE-roofline oriented):
  - Wq/Wk/Wv are cached in SBUF in bf16 for the whole of phase 1 (the
    v1 kernel re-streamed them every token slice: ~200 MB of HBM
    traffic became ~13 MB).
  - X^T streams through SBUF in bf16 chunks of 512 tokens.
  - Q^T/K^T (RoPE applied, fp32) go to DRAM scratch in fp32r so the
    phase-2 score matmuls keep fp32 precision; V goes in bf16.
  - The causal mask has only 4 distinct 128x512 diagonal block
    patterns; they are loaded once and reused (v1 streamed ~33 MB).
  - Softmax: exp on ACT (bf16 out), denominators via ones-matmul on PE,
    normalization on DVE. PSUM->SBUF copies ride the ACT engine.
  - 1/sqrt(HD) is folded into Wq on the host.
"""
import sys

sys.path.insert(0, "/opt/trn_rl_repo")

import numpy as np

import concourse.bass as bass
import concourse.bacc as bacc
import concourse.tile as tile
import concourse.mybir as mybir

B, S, H, NH = 2, 2048, 4096, 32
HD = H // NH          # 128
NC = 8                # cores
DL = H // NC          # 512 local dims (4 heads / core)
NHL = NH // NC        # 4 local heads
BT = B * S            # 4096 tokens
P = 128
CH = 512              # phase-1 token chunk (matmul moving dim)
QT = 512              # phase-2 query tile (free dim)
KT = 128              # phase-2 key tile (partition dim)
NKO = H // P          # 32 contraction subtiles

DT = mybir.dt.float32
DTR = mybir.dt.float32r
BF = mybir.dt.bfloat16
F32 = mybir.dt.float32
AF = mybir.ActivationFunctionType


def _phase1(nc, tc, pools, aps, scratches):
    """QKV projections + RoPE, all batches, W cached in SBUF."""
    px, pt, p1r, pvo, psA, psV = pools
    xt3, wq3, wk3, wv3, cos_ap, sin_ap, wq_tiles, wk_tiles, wv_tiles = aps

    # chunk-0 inputs first (startup-critical), then the weight caches in
    # first-use order so the first matmuls aren't queued behind ~13 MB
    xc0 = px.tile([P, NKO, CH], BF, tag="xt", name="xc0")
    nc.sync.dma_start(xc0[:], xt3[:, :, bass.ds(0, CH)])
    cos0 = pt.tile([P, CH], DT, tag="cos", name="cos0")
    nc.sync.dma_start(cos0[:], cos_ap[:, bass.ds(0, CH)])
    sin0 = pt.tile([P, CH], DT, tag="sin", name="sin0")
    nc.sync.dma_start(sin0[:], sin_ap[:, bass.ds(0, CH)])
    for dsub in range(DL // P):
        nc.sync.dma_start(wq_tiles[dsub][:], wq3[:, :, bass.ts(dsub, P)])
    for dsub in range(DL // P):
        nc.sync.dma_start(wk_tiles[dsub][:], wk3[:, :, bass.ts(dsub, P)])
    for g in range(4):
        nc.sync.dma_start(wv_tiles[g][:], wv3[:, bass.ds(g * 8, 8), :])

    for c in range(BT // CH):                      # 8 chunks of 512 tokens
        b, pos = c // (S // CH), c % (S // CH)
        qt_d, kt_d, v_d = scratches[b]
        if c == 0:
            xc, cosT, sinT = xc0, cos0, sin0
        else:
            xc = px.tile([P, NKO, CH], BF, tag="xt")
            nc.sync.dma_start(xc[:], xt3[:, :, bass.ds(c * CH, CH)])
            cosT = pt.tile([P, CH], DT, tag="cos")
            nc.sync.dma_start(cosT[:], cos_ap[:, bass.ds(pos * CH, CH)])
            sinT = pt.tile([P, CH], DT, tag="sin")
            nc.sync.dma_start(sinT[:], sin_ap[:, bass.ds(pos * CH, CH)])

        for (wt, outd) in ((wq_tiles, qt_d), (wk_tiles, kt_d)):
            for dsub in range(DL // P):
                psum = psA.tile([P, CH], F32, tag="qk")
                for hs in range(NKO):
                    nc.tensor.matmul(
                        psum[:], wt[dsub][:, hs, :], xc[:, hs, :],
                        start=(hs == 0), stop=(hs == NKO - 1))
                rc = p1r.tile([P, CH], DTR, tag="rc")
                rs = p1r.tile([P, CH], F32, tag="rs")
                nc.vector.tensor_mul(rc[:], psum[:], cosT[:])
                nc.vector.tensor_mul(
                    rs[0:64, :], psum[64:128, :], sinT[0:64, :])
                nc.vector.tensor_mul(
                    rs[64:128, :], psum[0:64, :], sinT[64:128, :])
                nc.vector.tensor_tensor(
                    rc[0:64, :], rc[0:64, :], rs[0:64, :],
                    mybir.AluOpType.subtract)
                nc.vector.tensor_tensor(
                    rc[64:128, :], rc[64:128, :], rs[64:128, :],
                    mybir.AluOpType.add)
                nc.sync.dma_start(
                    outd[bass.ts(dsub, P), bass.ds(pos * CH, CH)], rc[:])

        # V in [t, d] layout
        psums = [psV.tile([P, DL], F32, tag=f"v{j}", name=f"vps{j}")
                 for j in range(CH // P)]
        for hs in range(NKO):
            wv_sl = wv_tiles[hs // 8][:, hs % 8, :]
            for j in range(CH // P):
                nc.tensor.matmul(
                    psums[j][:], xc[:, hs, bass.ts(j, P)], wv_sl,
                    start=(hs == 0), stop=(hs == NKO - 1))
        for j in range(CH // P):
            vo = pvo.tile([P, DL], BF, tag="vo")
            nc.scalar.activation(vo[:], psums[j][:], AF.Copy)
            nc.sync.dma_start(
                v_d[bass.ds(pos * CH + j * P, P), :], vo[:])


def _phase2_batch(nc, tc, b, spec, pools, mask_sb, maskt, mb, ones_bf,
                  scratch, ctxT):
    """Attention for batch b -> ctxT [P, NHL, S]."""
    p2, p2e, p2m, psS, psSum, psC = pools
    qt_d, kt_d, v_d = scratch

    for h in range(NHL):
        # Pool-queue loads: prefetch out-of-band of the phase-1 SP stream
        k_sb = p2.tile([P, S], DTR, tag="k_sb")
        nc.gpsimd.dma_start(k_sb[:], kt_d[bass.ts(h, P), :])
        q_sb = p2.tile([P, S], DTR, tag="q_sb")
        nc.gpsimd.dma_start(q_sb[:], qt_d[bass.ts(h, P), :])
        v_sb = p2.tile([P, S // P, P], BF, tag="v_sb")
        nc.gpsimd.dma_start(
            v_sb[:], v_d[:, bass.ts(h, P)].rearrange("(kt p) d -> p kt d", p=P))
        for qt in range(S // QT):
            blocks = spec[qt]
            nb = len(blocks)
            psum_sum = psSum.tile([P, QT], F32, tag="sum")
            psum_ctx = psC.tile([P, QT], F32, tag="ctx")
            for bi, (kt, pat) in enumerate(blocks):
                psum_s = psS.tile([P, QT], F32, tag="s")
                nc.tensor.matmul(
                    psum_s[:], k_sb[:, bass.ts(kt, KT)],
                    q_sb[:, bass.ts(qt, QT)], start=True, stop=True)
                if pat is not None and pat >= 0:
                    nc.vector.tensor_tensor(
                        psum_s[:], psum_s[:], mask_sb[:, pat, :],
                        mybir.AluOpType.add)
                elif pat is not None:          # general (non-causal) block
                    mk = p2m.tile([P, QT], DT, tag="mk")
                    nc.sync.dma_start(
                        mk[:], maskt[mb, bass.ts(kt, KT), bass.ts(qt, QT)])
                    nc.vector.tensor_tensor(
                        psum_s[:], psum_s[:], mk[:], mybir.AluOpType.add)
                e_sb = p2e.tile([P, QT], BF, tag="e")
                nc.scalar.activation(e_sb[:], psum_s[:], AF.Exp)
                nc.tensor.matmul(psum_sum[:], ones_bf[:], e_sb[:],
                                 start=(bi == 0), stop=(bi == nb - 1))
                nc.tensor.matmul(psum_ctx[:], v_sb[:, kt, :], e_sb[:],
                                 start=(bi == 0), stop=(bi == nb - 1))
            recip = p2e.tile([P, QT], F32, tag="recip")
            nc.vector.reciprocal(recip[:], psum_sum[:])
            nc.vector.tensor_mul(
                ctxT[:, h, bass.ts(qt, QT)], psum_ctx[:], recip[:])


def _phase3(nc, tc, pools, wo3, ctx_tiles, ot):
    p3w, p3o, psO = pools
    for b in range(B):
        ctxT = ctx_tiles[b]
        for oi in range(H // P):
            wo_sb = p3w.tile([P, NHL, P], DTR, tag="wo")
            nc.gpsimd.dma_start(wo_sb[:], wo3[:, :, bass.ts(oi, P)])
            for qt in range(S // QT):
                psum_o = psO.tile([P, QT], F32, tag="o")
                for hs in range(NHL):
                    nc.tensor.matmul(
                        psum_o[:], wo_sb[:, hs, :], ctxT[:, hs, bass.ts(qt, QT)],
                        start=(hs == 0), stop=(hs == NHL - 1))
                o_sb = p3o.tile([P, QT], DT, tag="o_sb")
                nc.scalar.activation(o_sb[:], psum_o[:], AF.Copy)
                nc.sync.dma_start(
                    ot[bass.ts(oi, P), bass.ds(b * S + qt * QT, QT)], o_sb[:])


def _build(specs, n_mb, reps=1, phases=(1, 2, 3)):
    nc = bacc.Bacc()

    xt = nc.declare_dram_parameter("xt", [H, BT], BF, isOutput=False)
    wqt = nc.declare_dram_parameter("wqt", [H, DL], BF, isOutput=False)
    wkt = nc.declare_dram_parameter("wkt", [H, DL], BF, isOutput=False)
    wvt = nc.declare_dram_parameter("wvt", [H, DL], BF, isOutput=False)
    wot = nc.declare_dram_parameter("wot", [DL, H], DTR, isOutput=False)
    mask4 = nc.declare_dram_parameter("mask4", [4, KT, QT], DT, isOutput=False)
    maskt = nc.declare_dram_parameter("maskt", [n_mb, S, S], DT, isOutput=False)
    cos_p = nc.declare_dram_parameter("cos", [HD, S], DT, isOutput=False)
    sin_p = nc.declare_dram_parameter("sin", [HD, S], DT, isOutput=False)
    ot = nc.declare_dram_parameter("ot", [H, BT], DT, isOutput=True)

    xt3 = xt.rearrange("(ho p) t -> p ho t", p=P)
    wq3 = wqt.rearrange("(ho p) d -> p ho d", p=P)
    wk3 = wkt.rearrange("(ho p) d -> p ho d", p=P)
    wv3 = wvt.rearrange("(ho p) d -> p ho d", p=P)
    wo3 = wot.rearrange("(hs p) o -> p hs o", p=P)
    mask4r = mask4.rearrange("f p q -> p f q")

    import contextlib

    with tile.TileContext(nc) as tc:
        with (
            tc.tile_pool(name="glob", bufs=1) as glob,
            tc.tile_pool(name="dram", bufs=1, space="DRAM") as dram,
        ):
            scratches = []
            for b in range(B):
                qd = dram.tile([DL, S], DTR, tag=f"qt_d{b}", name=f"qt_d{b}")
                kd = dram.tile([DL, S], DTR, tag=f"kt_d{b}", name=f"kt_d{b}")
                vd = dram.tile([S, DL], BF, tag=f"v_d{b}", name=f"v_d{b}")
                scratches.append((qd, kd, vd))

            ones_f = glob.tile([P, P], F32, tag="ones_f")
            nc.any.memset(ones_f[:], 1.0)
            ones_bf = glob.tile([P, P], BF, tag="ones_bf")
            nc.vector.tensor_copy(ones_bf[:], ones_f[:])
            mask_sb = glob.tile([P, 4, QT], DT, tag="mask_sb")
            nc.gpsimd.dma_start(mask_sb[:], mask4r[:, :, :])

            loop_cm = tc.For_i(0, reps, 1) if reps > 1 else contextlib.nullcontext()
            with loop_cm:
                if 1 in phases:
                    with (
                        tc.tile_pool(name="pw", bufs=1) as pw,
                        tc.tile_pool(name="px", bufs=2) as px,
                        tc.tile_pool(name="pt", bufs=2) as pt,
                        tc.tile_pool(name="p1r", bufs=2) as p1r,
                        tc.tile_pool(name="pvo", bufs=3) as pvo,
                        tc.tile_pool(name="psA", bufs=3, space="PSUM") as psA,
                        tc.tile_pool(name="psV", bufs=1, space="PSUM") as psV,
                    ):
                        wq_tiles = [pw.tile([P, NKO, P], BF, tag=f"wq{d}", name=f"wq{d}")
                                    for d in range(DL // P)]
                        wk_tiles = [pw.tile([P, NKO, P], BF, tag=f"wk{d}", name=f"wk{d}")
                                    for d in range(DL // P)]
                        wv_tiles = [pw.tile([P, 8, DL], BF, tag=f"wv{g}", name=f"wv{g}")
                                    for g in range(4)]
                        aps = (xt3, wq3, wk3, wv3, cos_p, sin_p,
                               wq_tiles, wk_tiles, wv_tiles)
                        _phase1(nc, tc, (px, pt, p1r, pvo, psA, psV),
                                aps, scratches)
                if 2 in phases:
                    with tc.tile_pool(name="ctxp", bufs=1) as ctxp:
                        ctx_tiles = []
                        with (
                            tc.tile_pool(name="p2", bufs=2) as p2,
                            tc.tile_pool(name="p2e", bufs=3) as p2e,
                            tc.tile_pool(name="p2m", bufs=2) as p2m,
                            tc.tile_pool(name="psS", bufs=3, space="PSUM") as psS,
                            tc.tile_pool(name="psSum", bufs=2, space="PSUM") as psSum,
                            tc.tile_pool(name="psC", bufs=2, space="PSUM") as psC,
                        ):
                            for b in range(B):
                                mb = b % n_mb
                                ctxT = ctxp.tile([P, NHL, S], DTR, tag=f"ctxT{b}",
                                                 name=f"ctxT{b}")
                                ctx_tiles.append(ctxT)
                                _phase2_batch(
                                    nc, tc, b, specs[mb],
                                    (p2, p2e, p2m, psS, psSum, psC),
                                    mask_sb, maskt, mb, ones_bf,
                                    scratches[b], ctxT)
                        if 3 in phases:
                            with (
                                tc.tile_pool(name="p3w", bufs=3) as p3w,
                                tc.tile_pool(name="p3o", bufs=4) as p3o,
                                tc.tile_pool(name="psO", bufs=4, space="PSUM") as psO,
                            ):
                                _phase3(nc, tc, (p3w, p3o, psO), wo3, ctx_tiles, ot)
    nc.finalize()
    return nc


def _rope_tables():
    inv_freq = 1.0 / (10000.0 ** (np.arange(0, HD, 2, dtype=np.float32) / HD))
    t = np.arange(S, dtype=np.float32)
    freqs = np.einsum("i,j->ij", t, inv_freq)
    emb = np.concatenate([freqs, freqs], axis=-1)        # [S, HD]
    return np.cos(emb).astype(np.float32), np.sin(emb).astype(np.float32)


def _block_spec(mask):
    """mask: [S, S] additive (q, k). Per-qt list of (kt, pat):
    pat None = unmasked block, 0..3 = cached causal diagonal pattern,
    -1 = general masked block (loaded from maskt)."""
    pats = [np.ascontiguousarray(mask[0:QT, kl * KT:(kl + 1) * KT].T)
            for kl in range(QT // KT)]
    spec = []
    for qt in range(S // QT):
        row = []
        sub_q = mask[qt * QT:(qt + 1) * QT]
        for kt in range(S // KT):
            blk = sub_q[:, kt * KT:(kt + 1) * KT]
            if np.all(blk <= -1e8):
                continue                        # fully masked -> skip
            if not np.any(blk != 0.0):
                row.append((kt, None))
                continue
            kl = kt - qt * (QT // KT)
            if 0 <= kl < QT // KT and np.array_equal(blk.T, pats[kl]):
                row.append((kt, kl))
            else:
                row.append((kt, -1))
        assert row, "a query tile with all keys masked is not supported"
        spec.append(row)
    return spec, pats


_CACHE = {}


def _prepare(hidden_states, attention_mask, Wq, Wk, Wv, Wo):
    """Host-side marshaling -> (specs, n_mb, in_maps)."""
    import ml_dtypes

    bf16 = ml_dtypes.bfloat16
    hidden_states = np.asarray(hidden_states, dtype=np.float32)
    attention_mask = np.asarray(attention_mask, dtype=np.float32)
    Wq = np.asarray(Wq, dtype=np.float32)
    Wk = np.asarray(Wk, dtype=np.float32)
    Wv = np.asarray(Wv, dtype=np.float32)
    Wo = np.asarray(Wo, dtype=np.float32)

    xt = np.ascontiguousarray(
        hidden_states.reshape(BT, H).T).astype(bf16)            # [H, BT]
    scale = 1.0 / np.sqrt(np.float32(HD))
    wqT = np.ascontiguousarray(Wq.T * scale).astype(bf16)       # [H, H] (in, out)
    wkT = np.ascontiguousarray(Wk.T).astype(bf16)
    wvT = np.ascontiguousarray(Wv.T).astype(bf16)
    woT = np.ascontiguousarray(Wo.T)                            # [H(in'), H(out)]

    masks = attention_mask[:, 0]                                # [B, S, S]
    same = bool(np.array_equal(masks[0], masks[1])) if B == 2 else True
    n_mb = 1 if same else B
    sp = [_block_spec(masks[i]) for i in range(n_mb)]
    specs = [s for s, _ in sp]
    mask4 = np.ascontiguousarray(np.stack(sp[0][1]))            # [4, KT, QT]
    maskt = np.ascontiguousarray(
        np.stack([masks[i].T for i in range(n_mb)]))            # [n_mb, S(k), S(q)]

    cos, sin = _rope_tables()
    cos_t = np.ascontiguousarray(cos.T)                         # [HD, S]
    sin_t = np.ascontiguousarray(sin.T)

    in_maps = []
    for g in range(NC):
        dsl = slice(g * DL, (g + 1) * DL)
        in_maps.append({
            "xt": xt,
            "wqt": np.ascontiguousarray(wqT[:, dsl]),
            "wkt": np.ascontiguousarray(wkT[:, dsl]),
            "wvt": np.ascontiguousarray(wvT[:, dsl]),
            "wot": np.ascontiguousarray(woT[dsl, :]),
            "mask4": mask4,
            "maskt": maskt,
            "cos": cos_t, "sin": sin_t,
        })
    return specs, n_mb, in_maps


def kernel(hidden_states, attention_mask, Wq, Wk, Wv, Wo):
    from concourse.bass_utils import run_bass_kernel_spmd

    specs, n_mb, in_maps = _prepare(
        hidden_states, attention_mask, Wq, Wk, Wv, Wo)

    key = (n_mb, tuple(tuple(map(tuple, s)) for s in specs))
    if key not in _CACHE:
        _CACHE[key] = _build(specs, n_mb)
    nc = _CACHE[key]

    try:
        res = run_bass_kernel_spmd(nc, in_maps, list(range(NC)), trace=False)
    except Exception:
        # one retry: a wedged NeuronCore usually recovers on re-dispatch
        import time as _time
        _time.sleep(5)
        res = run_bass_kernel_spmd(nc, in_maps, list(range(NC)), trace=False)
    acc = np.zeros((H, BT), dtype=np.float32)
    for g in range(NC):
        acc += res.results[g]["ot"]
    return np.ascontiguousarray(acc.T).reshape(B, S, H)
